# revision 1
# baseline (speedup 1.0000x reference)
"""CapsuleLayer dynamic-routing kernel for one TRN2 chip (8 NeuronCores).

Strategy (per spec sharding_hint): data-parallel over the batch axis.
Each of the 8 cores gets B/8 = 32 samples; route_weights are replicated.
priors = einsum('bri,crio->cbro', x, W) is computed per-shard on-device,
followed by 3 routing iterations (softmax over routes / weighted sum /
squash) which are purely per-(cap, sample) — no cross-device traffic.

Shapes (hardcoded per problem spec nn_CapsuleLayer_8375186227255):
  x             [256, 1152, 8]   f32
  route_weights [10, 1152, 8, 16] f32
  output        [10, 256, 1, 1, 16] f32
"""

import numpy as np

B, R, CIN = 256, 1152, 8
NCAPS, COUT = 10, 16
N_CORES = 8
B_LOC = B // N_CORES
NUM_ITERATIONS = 3

_COMPILED = {}


def _routing_shard(x_s, w):
    """One-shard capsule routing. x_s [B_LOC,R,CIN], w [NCAPS,R,CIN,COUT]
    -> [NCAPS, B_LOC, COUT]."""
    import jax
    import jax.numpy as jnp

    # priors [c, b, r, o]
    priors = jnp.einsum("bri,crio->cbro", x_s, w)
    # logits are rank-degenerate over o (zeros + broadcast update), keep [c,b,r]
    logits = jnp.zeros(priors.shape[:3], dtype=priors.dtype)
    outputs = None
    for i in range(NUM_ITERATIONS):
        probs = jax.nn.softmax(logits, axis=2)  # over routes
        s = jnp.einsum("cbr,cbro->cbo", probs, priors)
        sq = jnp.sum(s * s, axis=-1, keepdims=True)
        outputs = (sq / (1.0 + sq)) * s / jnp.sqrt(sq)
        if i != NUM_ITERATIONS - 1:
            logits = logits + jnp.einsum("cbro,cbo->cbr", priors, outputs)
    return outputs


def _get_compiled():
    if "fn" not in _COMPILED:
        import jax

        _COMPILED["fn"] = jax.pmap(
            _routing_shard,
            axis_name="cores",
            in_axes=(0, 0),
            devices=jax.devices()[:N_CORES],
        )
    return _COMPILED["fn"]


def _replicated_weights(w: np.ndarray):
    """Device-resident replicated weights, cached across calls."""
    import hashlib

    import jax

    key = hashlib.sha1(w.tobytes()).hexdigest()
    if _COMPILED.get("w_key") != key:
        devs = jax.devices()[:N_CORES]
        _COMPILED["w_dev"] = jax.device_put_sharded([w] * N_CORES, devs)
        _COMPILED["w_key"] = key
    return _COMPILED["w_dev"]


def kernel(x: np.ndarray, route_weights: np.ndarray) -> np.ndarray:
    fn = _get_compiled()
    x = np.ascontiguousarray(x, dtype=np.float32).reshape(N_CORES, B_LOC, R, CIN)
    w = np.ascontiguousarray(route_weights, dtype=np.float32)
    out = np.asarray(fn(x, _replicated_weights(w)))  # [N_CORES, NCAPS, B_LOC, COUT]
    full = out.transpose(1, 0, 2, 3).reshape(NCAPS, B, COUT)
    return full.reshape(NCAPS, B, 1, 1, COUT).astype(np.float32)

